# revision 1
# baseline (speedup 1.0000x reference)
"""ChebNet (K=3, 3 layers) GNN on 8 Trainium2 NeuronCores.

Math: per layer, out = h@(W0-W2) + L(h@W1 + 2*L(h@W2)) + b, where
L(v) = -dis * S(dis * v), S = unweighted scatter-add over edges, and
dis = rsqrt(clamp(outdeg,1)) masked by outdeg>0.  The per-edge weight
w = dis[src]*dis[dst] folds into two dense per-node row scalings.

Sharding: nodes split into 8 contiguous ranges (one per core, padded to
49*128 rows).  Each core owns the edges whose dst falls in its range and
computes output rows for its range only.  Before each graph op (lap) the
dis-scaled operand is AllGathered so every core can gather arbitrary src
rows with dma_gather.  dma_gather indices are int16, so the 50176-row
gathered tensor is addressed through two windows (rows [0,32768) and
[32768,50176)); each dst-tile's edges are split lo/hi by src window and
the two partial scatter sums merge for free in PSUM accumulation.

Scatter-add is done on the tensor engine: for each 128-edge chunk, a
[128e x 128dst] one-hot matrix (built on the vector engine by comparing
an iota row against the per-edge local dst) is matmul'd against the
gathered [128e x 64f] rows, accumulating [128dst x 64f] in PSUM.
"""

import sys

sys.path.insert(0, "/opt/trn_rl_repo")

import numpy as np
from contextlib import ExitStack

_REAL = dict(N=50000, E=800000, NCORES=8, LO=32768, F0=128, F1=64, F2=16)


# ---------------------------------------------------------------- host prep
def _derive(cfg):
    c = dict(cfg)
    c["NPC"] = c["N"] // c["NCORES"]
    c["NT"] = -(-c["NPC"] // 128)
    c["NPAD"] = c["NT"] * 128
    c["NG"] = c["NCORES"] * c["NPAD"]
    c["FW"] = 64  # lap working width (256B gather elements)
    assert c["LO"] <= 32768 and c["NG"] - c["LO"] <= 32768
    assert c["N"] % c["NCORES"] == 0
    return c


def _prep(edge_index, c):
    N, E, NCORES, LO = c["N"], c["E"], c["NCORES"], c["LO"]
    NPC, NT, NPAD = c["NPC"], c["NT"], c["NPAD"]

    src = np.asarray(edge_index[0], dtype=np.int64)
    dst = np.asarray(edge_index[1], dtype=np.int64)
    assert src.shape == (E,) and dst.shape == (E,)
    psrc = (src // NPC) * NPAD + (src % NPC)  # padded global row of src

    cd = dst // NPC
    ld = dst - cd * NPC
    td = ld >> 7
    dl = ld & 127
    hi = (psrc >= LO).astype(np.int64)

    # ---- lap tables: edges grouped by (core, dst-tile, window), src-sorted
    counts = np.zeros((NCORES, NT, 2), np.int64)
    np.add.at(counts, (cd, td, hi), 1)
    Klo = np.maximum(1, -(-counts[:, :, 0].max(0) // 128))
    Khi = np.maximum(1, -(-counts[:, :, 1].max(0) // 128))
    LOFF = np.concatenate([[0], np.cumsum(Klo)]).astype(np.int64)
    HOFF = np.concatenate([[0], np.cumsum(Khi)]).astype(np.int64)
    TLO, THI = int(LOFF[-1]), int(HOFF[-1])

    order = np.lexsort((psrc, hi, td, cd))
    cd_s, td_s, hi_s = cd[order], td[order], hi[order]
    dl_s, psrc_s = dl[order], psrc[order]
    grp = (cd_s * NT + td_s) * 2 + hi_s
    gc = np.bincount(grp, minlength=NCORES * NT * 2)
    gstart = np.concatenate([[0], np.cumsum(gc)])[:-1]
    rank = np.arange(E) - gstart[grp]

    gidx_lo = np.zeros((NCORES, TLO * 128), np.int16)
    gidx_hi = np.zeros((NCORES, THI * 128), np.int16)
    dloc_lo = np.full((NCORES, 128, TLO), -1.0, np.float32)
    dloc_hi = np.full((NCORES, 128, THI), -1.0, np.float32)
    for cc in range(NCORES):
        for h, (gidx, dloc, OFF, base) in enumerate(
            [(gidx_lo, dloc_lo, LOFF, 0), (gidx_hi, dloc_hi, HOFF, LO)]
        ):
            m = (cd_s == cc) & (hi_s == h)
            slot = OFF[td_s[m]] + rank[m] // 128
            part = rank[m] & 127
            gidx[cc, slot * 128 + part] = (psrc_s[m] - base).astype(np.int16)
            dloc[cc, part, slot] = dl_s[m].astype(np.float32)

    # ---- degree tables: edges grouped by (core, src-tile)
    cs = src // NPC
    ls = src - cs * NPC
    ts_ = ls >> 7
    sl = ls & 127
    dcounts = np.zeros((NCORES, NT), np.int64)
    np.add.at(dcounts, (cs, ts_), 1)
    Kd = np.maximum(1, -(-dcounts.max(0) // 128))
    SOFF = np.concatenate([[0], np.cumsum(Kd)]).astype(np.int64)
    TD = int(SOFF[-1])

    order2 = np.lexsort((ts_, cs))
    cs_s, tt_s, sl_s = cs[order2], ts_[order2], sl[order2]
    grp2 = cs_s * NT + tt_s
    gc2 = np.bincount(grp2, minlength=NCORES * NT)
    gstart2 = np.concatenate([[0], np.cumsum(gc2)])[:-1]
    rank2 = np.arange(E) - gstart2[grp2]
    sloc = np.full((NCORES, 128, TD), -1.0, np.float32)
    for cc in range(NCORES):
        m = cs_s == cc
        slot = SOFF[tt_s[m]] + rank2[m] // 128
        sloc[cc, rank2[m] & 127, slot] = sl_s[m].astype(np.float32)

    def wrap(a):  # int16 [M*128] -> [128, M*8], idx j at [j%16, j//16], x8 replicated
        return np.tile(a.reshape(-1, 16).T, (8, 1)).copy()

    return dict(
        Klo=Klo, Khi=Khi, Kd=Kd, LOFF=LOFF, HOFF=HOFF, SOFF=SOFF,
        TLO=TLO, THI=THI, TD=TD,
        gidx_lo=[wrap(gidx_lo[cc]) for cc in range(NCORES)],
        gidx_hi=[wrap(gidx_hi[cc]) for cc in range(NCORES)],
        dloc_lo=dloc_lo, dloc_hi=dloc_hi, sloc=sloc,
    )


# ---------------------------------------------------------------- device build
def _build(c, pp, Fins, use_bias, stages=99):
    import concourse.bacc as bacc
    import concourse.tile as tile
    from concourse import mybir

    f32, i16 = mybir.dt.float32, mybir.dt.int16
    AOT = mybir.AluOpType
    NT, NPAD, NG, LO, FW = c["NT"], c["NPAD"], c["NG"], c["LO"], c["FW"]
    NCORES, F0, F2 = c["NCORES"], c["F0"], c["F2"]
    TLO, THI, TD = pp["TLO"], pp["THI"], pp["TD"]
    Klo, Khi, Kd = pp["Klo"], pp["Khi"], pp["Kd"]
    LOFF, HOFF, SOFF = pp["LOFF"], pp["HOFF"], pp["SOFF"]
    GRP = 4
    groups = [list(range(g * GRP, min((g + 1) * GRP, NT))) for g in range(-(-NT // GRP))]

    nc = bacc.Bacc(num_devices=NCORES)

    xin = nc.dram_tensor("x", [NPAD, F0], f32, kind="ExternalInput")
    gl_d = nc.dram_tensor("gidx_lo", [128, TLO * 8], i16, kind="ExternalInput")
    gh_d = nc.dram_tensor("gidx_hi", [128, THI * 8], i16, kind="ExternalInput")
    dl_d = nc.dram_tensor("dloc_lo", [128, TLO], f32, kind="ExternalInput")
    dh_d = nc.dram_tensor("dloc_hi", [128, THI], f32, kind="ExternalInput")
    sl_d = nc.dram_tensor("sloc", [128, TD], f32, kind="ExternalInput")
    iota_d = nc.dram_tensor("iota", [128, 128], f32, kind="ExternalInput")
    id_d = nc.dram_tensor("ident", [128, 128], f32, kind="ExternalInput")
    W_d = {}
    for l in range(3):
        for nm in ("wa", "wb", "wc"):
            W_d[nm, l] = nc.dram_tensor(f"{nm}{l}", [Fins[l], FW], f32, kind="ExternalInput")
        if use_bias[l]:
            W_d["br", l] = nc.dram_tensor(f"br{l}", [128, FW], f32, kind="ExternalInput")
    y_d = nc.dram_tensor("y", [NPAD, F2], f32, kind="ExternalOutput")

    ag_in = [nc.dram_tensor(f"agin{i}", [NPAD, FW], f32) for i in range(6)]
    ag_out = [nc.dram_tensor(f"agout{i}", [NG, FW], f32, addr_space="Shared") for i in range(6)]

    xv = xin.rearrange("(t p) f -> p t f", p=128)
    yv = y_d.rearrange("(t p) f -> p t f", p=128)

    with tile.TileContext(nc) as tc, ExitStack() as ctx:
        cst = ctx.enter_context(tc.tile_pool(name="cst", bufs=1))
        big = ctx.enter_context(tc.tile_pool(name="big", bufs=1))
        gp = ctx.enter_context(tc.tile_pool(name="gp", bufs=2))
        ohp = ctx.enter_context(tc.tile_pool(name="ohp", bufs=6))
        smp = ctx.enter_context(tc.tile_pool(name="smp", bufs=6))
        slb = ctx.enter_context(tc.tile_pool(name="slb", bufs=2))
        psA = ctx.enter_context(tc.tile_pool(name="psA", bufs=2, space="PSUM"))
        psT = ctx.enter_context(tc.tile_pool(name="psT", bufs=2, space="PSUM"))
        psD = ctx.enter_context(tc.tile_pool(name="psD", bufs=2, space="PSUM"))

        # constants
        iota = cst.tile([128, 128], f32)
        nc.sync.dma_start(iota[:], iota_d[:])
        ident = cst.tile([128, 128], f32)
        nc.sync.dma_start(ident[:], id_d[:])
        ones = cst.tile([128, 1], f32)
        nc.vector.memset(ones[:], 1.0)
        gl = cst.tile([128, TLO * 8], i16)
        nc.sync.dma_start(gl[:], gl_d[:])
        gh = cst.tile([128, THI * 8], i16)
        nc.sync.dma_start(gh[:], gh_d[:])
        dlo = cst.tile([128, TLO], f32)
        nc.sync.dma_start(dlo[:], dl_d[:])
        dhi = cst.tile([128, THI], f32)
        nc.sync.dma_start(dhi[:], dh_d[:])
        slo = cst.tile([128, TD], f32)
        nc.sync.dma_start(slo[:], sl_d[:])
        Wt = {}
        for k, d in W_d.items():
            Wt[k] = cst.tile([128, FW], f32, name=f"w_{k[0]}_{k[1]}", tag=f"w_{k[0]}_{k[1]}")
            nc.sync.dma_start(Wt[k][: d.shape[0], :], d[:])
        dis = cst.tile([128, NT], f32)
        negdis = cst.tile([128, NT], f32)
        n2dis2 = cst.tile([128, NT], f32)

        # ---------------- degree -> dis tables
        for t in range(NT):
            acc = psA.tile([128, FW], f32, tag="acc")
            for k in range(int(Kd[t])):
                s = int(SOFF[t]) + k
                oh = ohp.tile([128, 128], f32, tag="oh")
                nc.vector.tensor_scalar(oh[:], iota[:], slo[:, s : s + 1], None, AOT.is_equal)
                nc.tensor.matmul(
                    acc[:, :1], oh[:], ones[:],
                    start=(k == 0), stop=(k == int(Kd[t]) - 1),
                )
            m = smp.tile([128, 1], f32, tag="m")
            nc.vector.tensor_scalar_max(m[:], acc[:, :1], 1.0)
            sq = smp.tile([128, 1], f32, tag="sq")
            nc.scalar.sqrt(sq[:], m[:])
            r = smp.tile([128, 1], f32, tag="r")
            nc.vector.reciprocal(r[:], sq[:])
            msk = smp.tile([128, 1], f32, tag="msk")
            nc.vector.tensor_scalar(msk[:], acc[:, :1], 0.0, None, AOT.is_gt)
            nc.vector.tensor_mul(dis[:, t : t + 1], r[:], msk[:])
            nc.vector.tensor_scalar_mul(negdis[:, t : t + 1], dis[:, t : t + 1], -1.0)
            d2 = smp.tile([128, 1], f32, tag="d2")
            nc.vector.tensor_mul(d2[:], dis[:, t : t + 1], dis[:, t : t + 1])
            nc.vector.tensor_scalar_mul(n2dis2[:, t : t + 1], d2[:], -2.0)

        # ---------------- lap helper
        import os as _os
        agmode = _os.environ.get("KAG", "cc")
        lapmode = _os.environ.get("KLAP", "full")
        dummy = cst.tile([128, FW], f32, name="dummy", tag="dummy")
        nc.vector.memset(dummy[:], 0.0)

        def lap(agi, epi):
            """Gather rows of ag_out[agi] per edge, scatter-add per dst tile,
            call epi(t, acc_psum) with the [128,FW] PSUM partial sums."""
            if lapmode == "skip":
                for t in range(NT):
                    acc = psA.tile([128, FW], f32, tag="acc")
                    oh = ohp.tile([128, 128], f32, tag="oh")
                    nc.vector.tensor_scalar(oh[:], iota[:], dlo[:, 0:1], None, AOT.is_equal)
                    nc.tensor.matmul(acc[:], oh[:], dummy[:], start=True, stop=True)
                    epi(t, acc)
                return
            src_lo = ag_out[agi][0:LO, :]
            src_hi = ag_out[agi][LO:NG, :]
            for tl in groups:
                a_lo, b_lo = int(LOFF[tl[0]]), int(LOFF[tl[-1] + 1])
                a_hi, b_hi = int(HOFF[tl[0]]), int(HOFF[tl[-1] + 1])
                nlo, nhi = b_lo - a_lo, b_hi - a_hi
                CAP = 8  # max 128-chunks (1024 idx) per dma_gather call
                glo = gp.tile([128, nlo, FW], f32, tag="glo")
                for o in range(0, nlo, CAP):
                    n = min(CAP, nlo - o)
                    nc.gpsimd.dma_gather(
                        glo[:, o : o + n, :], src_lo,
                        gl[:, (a_lo + o) * 8 : (a_lo + o + n) * 8],
                        num_idxs=n * 128, num_idxs_reg=n * 128, elem_size=FW,
                    )
                ghi_t = gp.tile([128, nhi, FW], f32, tag="ghi")
                for o in range(0, nhi, CAP):
                    n = min(CAP, nhi - o)
                    nc.gpsimd.dma_gather(
                        ghi_t[:, o : o + n, :], src_hi,
                        gh[:, (a_hi + o) * 8 : (a_hi + o + n) * 8],
                        num_idxs=n * 128, num_idxs_reg=n * 128, elem_size=FW,
                    )
                if lapmode == "gatheronly":
                    for t in tl:
                        acc = psA.tile([128, FW], f32, tag="acc")
                        oh = ohp.tile([128, 128], f32, tag="oh")
                        nc.vector.tensor_scalar(oh[:], iota[:], dlo[:, 0:1], None, AOT.is_equal)
                        nc.tensor.matmul(acc[:], oh[:], dummy[:], start=True, stop=True)
                        epi(t, acc)
                    continue
                for t in tl:
                    acc = psA.tile([128, FW], f32, tag="acc")
                    ntot = int(Klo[t]) + int(Khi[t])
                    i = 0
                    for k in range(int(Klo[t])):
                        s = int(LOFF[t]) + k
                        oh = ohp.tile([128, 128], f32, tag="oh")
                        nc.vector.tensor_scalar(
                            oh[:], iota[:], dlo[:, s : s + 1], None, AOT.is_equal
                        )
                        nc.tensor.matmul(
                            acc[:], oh[:], glo[:, s - a_lo, :],
                            start=(i == 0), stop=(i == ntot - 1),
                        )
                        i += 1
                    for k in range(int(Khi[t])):
                        s = int(HOFF[t]) + k
                        oh = ohp.tile([128, 128], f32, tag="oh")
                        nc.vector.tensor_scalar(
                            oh[:], iota[:], dhi[:, s : s + 1], None, AOT.is_equal
                        )
                        nc.tensor.matmul(
                            acc[:], oh[:], ghi_t[:, s - a_hi, :],
                            start=(i == 0), stop=(i == ntot - 1),
                        )
                        i += 1
                    epi(t, acc)

        # ---------------- layers
        h_prev = None
        nlayers = 3 if stages >= 99 else max(0, min(3, stages - 1))
        do_lap = stages >= 3 or stages >= 99
        for l in range(nlayers):
            Fin = Fins[l]
            As = big.tile([128, NT, FW], f32, tag="As")
            Cs1 = big.tile([128, NT, FW], f32, tag="Cs1")
            Oa = big.tile([128, NT, FW], f32, tag="Oa")
            for tl in groups:
                w = len(tl) * 128
                hT = slb.tile([128, GRP * 128], f32, tag="hT")
                hsT = slb.tile([128, GRP * 128], f32, tag="hsT")
                for u, t in enumerate(tl):
                    if l == 0:
                        ht = smp.tile([128, F0], f32, tag="xt")
                        nc.sync.dma_start(ht[:], xv[:, t, :])
                        ht_ap = ht[:]
                    else:
                        ht_ap = h_prev[:, t, :]
                    ps = psT.tile([128, 128], f32, tag="pt")
                    nc.tensor.transpose(ps[:Fin, :], ht_ap, ident[:])
                    nc.scalar.copy(hT[:Fin, u * 128 : (u + 1) * 128], ps[:Fin, :])
                    hs = smp.tile([128, Fin], f32, tag="hs")
                    nc.vector.tensor_scalar_mul(hs[:], ht_ap, dis[:, t : t + 1])
                    ps2 = psT.tile([128, 128], f32, tag="pt")
                    nc.tensor.transpose(ps2[:Fin, :], hs[:], ident[:])
                    nc.scalar.copy(hsT[:Fin, u * 128 : (u + 1) * 128], ps2[:Fin, :])
                for dstbuf, wkey, srcT in (
                    (As, ("wc", l), hsT),
                    (Cs1, ("wb", l), hsT),
                    (Oa, ("wa", l), hT),
                ):
                    pd = psD.tile([64, GRP * 128], f32, tag="pd")
                    nc.tensor.matmul(pd[:, :w], Wt[wkey][:Fin, :], srcT[:Fin, :w])
                    pT = slb.tile([64, GRP * 128], f32, tag="pT")
                    nc.scalar.copy(pT[:, :w], pd[:, :w])
                    for u, t in enumerate(tl):
                        pb = psT.tile([128, 128], f32, tag="pt")
                        nc.tensor.transpose(
                            pb[:, :FW], pT[:FW, u * 128 : (u + 1) * 128], ident[:FW, :FW]
                        )
                        nc.scalar.copy(dstbuf[:, t, :], pb[:, :FW])

            if not do_lap:
                h_prev = As
                continue
            agA = 2 * l
            nc.sync.dma_start(ag_in[agA].rearrange("(t p) f -> p t f", p=128), As[:])
            if agmode == "cc":
                nc.gpsimd.collective_compute(
                    "AllGather", mybir.AluOpType.bypass,
                    replica_groups=[list(range(NCORES))],
                    ins=[ag_in[agA][:, :]], outs=[ag_out[agA][:, :]],
                )
            else:
                nc.sync.dma_start(ag_out[agA][0:NPAD, :], ag_in[agA][:, :])

            Cs = big.tile([128, NT, FW], f32, tag="Cs")

            def epi1(t, acc):
                tmp = smp.tile([128, FW], f32, tag="t1")
                nc.vector.tensor_scalar_mul(tmp[:], acc[:], n2dis2[:, t : t + 1])
                nc.vector.tensor_add(Cs[:, t, :], Cs1[:, t, :], tmp[:])

            lap(agA, epi1)

            agC = 2 * l + 1
            nc.sync.dma_start(ag_in[agC].rearrange("(t p) f -> p t f", p=128), Cs[:])
            if agmode == "cc":
                nc.gpsimd.collective_compute(
                    "AllGather", mybir.AluOpType.bypass,
                    replica_groups=[list(range(NCORES))],
                    ins=[ag_in[agC][:, :]], outs=[ag_out[agC][:, :]],
                )
            else:
                nc.sync.dma_start(ag_out[agC][0:NPAD, :], ag_in[agC][:, :])

            hn = big.tile([128, NT, FW], f32, tag=f"h{l % 2}")

            def epi2(t, acc):
                tmp = smp.tile([128, FW], f32, tag="t1")
                nc.vector.tensor_scalar_mul(tmp[:], acc[:], negdis[:, t : t + 1])
                if use_bias[l]:
                    tmp2 = smp.tile([128, FW], f32, tag="t2")
                    nc.vector.tensor_add(tmp2[:], tmp[:], Oa[:, t, :])
                    pre = smp.tile([128, FW], f32, tag="t3")
                    nc.vector.tensor_add(pre[:], tmp2[:], Wt["br", l][:, :])
                else:
                    pre = smp.tile([128, FW], f32, tag="t2")
                    nc.vector.tensor_add(pre[:], tmp[:], Oa[:, t, :])
                if l < 2:
                    nc.vector.tensor_scalar_max(hn[:, t, :], pre[:], 0.0)
                else:
                    nc.vector.tensor_copy(hn[:, t, :], pre[:])

            lap(agC, epi2)
            h_prev = hn

        if h_prev is not None:
            nc.sync.dma_start(yv[:], h_prev[:, :, :F2])
        else:
            zt = big.tile([128, NT, FW], f32, tag="zt")
            nc.vector.memset(zt[:], 0.0)
            nc.sync.dma_start(yv[:], zt[:, :, :F2])

    nc.compile()
    return nc


# ---------------------------------------------------------------- entry
def _run(x, edge_index, Ws, bs, cfg=None, trace=False):
    from concourse.bass_utils import run_bass_kernel_spmd

    c = _derive(cfg or _REAL)
    N, NCORES, NPC, NPAD = c["N"], c["NCORES"], c["NPC"], c["NPAD"]
    F0, F2, FW = c["F0"], c["F2"], c["FW"]

    x = np.ascontiguousarray(np.asarray(x, dtype=np.float32))
    pp = _prep(edge_index, c)

    Fins = [F0, c["F1"], c["F1"]]
    use_bias = [bool(np.any(b)) for b in bs]
    nc = _build(c, pp, Fins, use_bias, stages=int(__import__('os').environ.get('KSTAGES', '99')))

    iota = np.tile(np.arange(128, dtype=np.float32), (128, 1))
    ident = np.eye(128, dtype=np.float32)

    def padW(w, fin):
        out = np.zeros((fin, FW), np.float32)
        out[: w.shape[0], : w.shape[1]] = w
        return out

    base = {"iota": iota, "ident": ident}
    for l in range(3):
        W = np.asarray(Ws[l], dtype=np.float32)
        base[f"wa{l}"] = padW(W[0] - W[2], Fins[l])
        base[f"wb{l}"] = padW(W[1], Fins[l])
        base[f"wc{l}"] = padW(W[2], Fins[l])
        if use_bias[l]:
            br = np.zeros((128, FW), np.float32)
            br[:, : bs[l].shape[0]] = np.asarray(bs[l], np.float32)
            base[f"br{l}"] = br

    in_maps = []
    for cc in range(NCORES):
        xl = np.zeros((NPAD, F0), np.float32)
        xl[:NPC] = x[cc * NPC : (cc + 1) * NPC]
        in_maps.append(
            dict(
                base,
                x=xl,
                gidx_lo=pp["gidx_lo"][cc],
                gidx_hi=pp["gidx_hi"][cc],
                dloc_lo=np.ascontiguousarray(pp["dloc_lo"][cc]),
                dloc_hi=np.ascontiguousarray(pp["dloc_hi"][cc]),
                sloc=np.ascontiguousarray(pp["sloc"][cc]),
            )
        )

    res = run_bass_kernel_spmd(nc, in_maps, core_ids=list(range(NCORES)), trace=trace)
    out = np.concatenate([res.results[cc]["y"][:NPC] for cc in range(NCORES)], axis=0)
    return out[:, :F2], res


def kernel(x, edge_index, W1, b1, Wm, bm, W2, b2):
    out, _ = _run(
        np.asarray(x), np.asarray(edge_index),
        [np.asarray(W1), np.asarray(Wm), np.asarray(W2)],
        [np.asarray(b1), np.asarray(bm), np.asarray(b2)],
    )
    return out



# revision 12
# speedup vs baseline: 1.0853x; 1.0853x over previous
"""ChebNet (K=3, 3 layers) GNN on 8 Trainium2 NeuronCores — v2.

Math per layer: out = h@(W0-W2) + L(h@W1 + 2*L(h@W2)) + b, where
L(v) = -dis * S(dis * v), S = unweighted scatter-add over edges, and
dis = rsqrt(clamp(deg,1)) masked by deg>0 (computed on HOST).

v2 design (vs the one-hot/f32 baseline):
- Feature-major state: h lives as hT [F, nodes] in SBUF; dense GEMMs use
  W as stationary and hT as moving (N=512) with zero transposes; only the
  49 per-tile pre-AllGather transposes remain (PE).
- bf16 everywhere on the lap path: the AllGather table is [node, 128] bf16
  with the 64 features DUPLICATED (256 B rows - dma_gather minimum elem).
- One-hots built in bf16: DVE tensor_scalar is_equal runs in 4x mode
  (~190 ns vs 1476 ns f32 measured); a fraction is built on the idle
  Scalar engine via square/relu trick to overlap.
- Scatter matmul per 128-edge chunk: stationary = gathered rows
  [128e, 64] bf16, moving = one-hot [128e, 128dst] bf16, accumulating
  accT [64f, 128dst] in PSUM (feature-major output, no transpose back).
- Epilogues scale acc by prebuilt per-column tables (negdisT / n2dis2T,
  broadcast down feature partitions, bf16, host-supplied).
- Gathers issued as 2048-index dma_gather calls, many in flight (pool
  bufs), so SDMA transfer overlaps DVE/ACT/PE chunk work.
"""

import os
import sys

sys.path.insert(0, "/opt/trn_rl_repo")

import numpy as np
from contextlib import ExitStack

_REAL = dict(N=50000, E=800000, NCORES=8, LO=32768, F0=128, FW=64, F2=16)


# ---------------------------------------------------------------- host prep
def _derive(cfg):
    c = dict(cfg)
    c["NPC"] = c["N"] // c["NCORES"]
    c["NT"] = -(-c["NPC"] // 128)
    c["NPAD"] = c["NT"] * 128
    c["NG"] = c["NCORES"] * c["NPAD"]
    c["W"] = c["NT"] * 128
    assert c["LO"] <= 32768 and c["NG"] - c["LO"] <= 32768
    assert c["N"] % c["NCORES"] == 0
    return c


def _prep(edge_index, c):
    from ml_dtypes import bfloat16

    N, E, NCORES, LO = c["N"], c["E"], c["NCORES"], c["LO"]
    NPC, NT, NPAD = c["NPC"], c["NT"], c["NPAD"]

    src = np.asarray(edge_index[0], dtype=np.int64)
    dst = np.asarray(edge_index[1], dtype=np.int64)
    assert src.shape == (E,) and dst.shape == (E,)
    psrc = (src // NPC) * NPAD + (src % NPC)  # padded global row of src

    # ---- degrees -> dis on host (reference: deg = segment_sum(ones, src))
    deg = np.bincount(src, minlength=N).astype(np.float32)
    dis = np.where(deg > 0, 1.0 / np.sqrt(np.maximum(deg, 1.0)), 0.0).astype(
        np.float32
    )
    dis_pad = np.zeros((NCORES, NPAD), np.float32)
    for cc in range(NCORES):
        dis_pad[cc, :NPC] = dis[cc * NPC : (cc + 1) * NPC]

    cd = dst // NPC
    ld = dst - cd * NPC
    td = ld >> 7
    dl = ld & 127
    hi = (psrc >= LO).astype(np.int64)

    # ---- lap tables: edges grouped by (core, dst-tile, window), src-sorted
    counts = np.zeros((NCORES, NT, 2), np.int64)
    np.add.at(counts, (cd, td, hi), 1)
    Klo = np.maximum(1, -(-counts[:, :, 0].max(0) // 128))
    Khi = np.maximum(1, -(-counts[:, :, 1].max(0) // 128))
    LOFF = np.concatenate([[0], np.cumsum(Klo)]).astype(np.int64)
    HOFF = np.concatenate([[0], np.cumsum(Khi)]).astype(np.int64)
    TLO, THI = int(LOFF[-1]), int(HOFF[-1])

    order = np.lexsort((psrc, hi, td, cd))
    cd_s, td_s, hi_s = cd[order], td[order], hi[order]
    dl_s, psrc_s = dl[order], psrc[order]
    grp = (cd_s * NT + td_s) * 2 + hi_s
    gc = np.bincount(grp, minlength=NCORES * NT * 2)
    gstart = np.concatenate([[0], np.cumsum(gc)])[:-1]
    rank = np.arange(E) - gstart[grp]

    gidx_lo = np.zeros((NCORES, TLO * 128), np.int16)
    gidx_hi = np.zeros((NCORES, THI * 128), np.int16)
    dloc_lo = np.full((NCORES, 128, TLO), -1.0, np.float32)
    dloc_hi = np.full((NCORES, 128, THI), -1.0, np.float32)
    for cc in range(NCORES):
        for h, (gidx, dloc, OFF, base) in enumerate(
            [(gidx_lo, dloc_lo, LOFF, 0), (gidx_hi, dloc_hi, HOFF, LO)]
        ):
            m = (cd_s == cc) & (hi_s == h)
            slot = OFF[td_s[m]] + rank[m] // 128
            part = rank[m] & 127
            gidx[cc, slot * 128 + part] = (psrc_s[m] - base).astype(np.int16)
            dloc[cc, part, slot] = dl_s[m].astype(np.float32)

    def wrap(a):  # int16 [M*128] -> [128, M*8], idx j at [j%16, j//16], x8 replicated
        return np.tile(a.reshape(-1, 16).T, (8, 1)).copy()

    return dict(
        Klo=Klo, Khi=Khi, LOFF=LOFF, HOFF=HOFF, TLO=TLO, THI=THI,
        gidx_lo=[wrap(gidx_lo[cc]) for cc in range(NCORES)],
        gidx_hi=[wrap(gidx_hi[cc]) for cc in range(NCORES)],
        dloc_lo=dloc_lo, dloc_hi=dloc_hi,
        dis_pad=dis_pad,
    )


# ---------------------------------------------------------------- device build
def _build(c, pp, Fins, use_bias):
    import concourse.bacc as bacc
    import concourse.tile as tile
    from concourse import mybir

    f32, i16, bf16 = mybir.dt.float32, mybir.dt.int16, mybir.dt.bfloat16
    AOT = mybir.AluOpType
    ACTF = mybir.ActivationFunctionType
    NT, NPAD, NG, LO, W = c["NT"], c["NPAD"], c["NG"], c["LO"], c["W"]
    NCORES, F0, FW, F2 = c["NCORES"], c["F0"], c["FW"], c["F2"]
    TLO, THI = pp["TLO"], pp["THI"]
    Klo, Khi = pp["Klo"], pp["Khi"]
    LOFF, HOFF = pp["LOFF"], pp["HOFF"]
    GRP = 4
    groups = [list(range(g * GRP, min((g + 1) * GRP, NT))) for g in range(-(-NT // GRP))]
    CAP = int(os.environ.get("KCAP", "8"))  # 128-idx chunks per call (1024 = ucode max)
    ACT_FRAC = int(os.environ.get("KACT", "4"))  # 1-in-N one-hots built on ScalarE

    nc = bacc.Bacc(num_devices=NCORES)

    xT_d = nc.dram_tensor("xT", [F0, W], bf16, kind="ExternalInput")
    gl_d = nc.dram_tensor("gidx_lo", [128, TLO * 8], i16, kind="ExternalInput")
    gh_d = nc.dram_tensor("gidx_hi", [128, THI * 8], i16, kind="ExternalInput")
    dl_d = nc.dram_tensor("dloc_lo", [128, TLO], f32, kind="ExternalInput")
    dh_d = nc.dram_tensor("dloc_hi", [128, THI], f32, kind="ExternalInput")
    iota_d = nc.dram_tensor("iota", [128, 128], bf16, kind="ExternalInput")
    id_d = nc.dram_tensor("ident", [128, 128], bf16, kind="ExternalInput")
    disT_d = nc.dram_tensor("disT", [128, W], bf16, kind="ExternalInput")
    nd_d = nc.dram_tensor("negdisT", [64, W], bf16, kind="ExternalInput")
    n2_d = nc.dram_tensor("n2dis2T", [64, W], bf16, kind="ExternalInput")
    W_d = {}
    for l in range(3):
        for nm in ("wa", "wb", "wc"):
            W_d[nm, l] = nc.dram_tensor(f"{nm}{l}", [Fins[l], FW], bf16,
                                        kind="ExternalInput")
        if use_bias[l]:
            W_d["br", l] = nc.dram_tensor(f"br{l}", [64, 1], bf16,
                                          kind="ExternalInput")
    y_d = nc.dram_tensor("y", [NPAD, F2], f32, kind="ExternalOutput")
    yv = y_d.rearrange("(t p) f -> p t f", p=128)

    ag_in = [nc.dram_tensor(f"agin{i}", [NPAD, 128], bf16) for i in range(6)]
    ag_out = [nc.dram_tensor(f"agout{i}", [NG, 128], bf16, addr_space="Shared")
              for i in range(6)]
    agmode = os.environ.get("KAG", "cc")

    with tile.TileContext(nc) as tc, ExitStack() as ctx:
        cst = ctx.enter_context(tc.tile_pool(name="cst", bufs=1))
        big = ctx.enter_context(tc.tile_pool(name="big", bufs=1))
        stg = ctx.enter_context(tc.tile_pool(name="stg", bufs=2))
        gp = ctx.enter_context(tc.tile_pool(name="gp", bufs=2))
        ohp = ctx.enter_context(tc.tile_pool(name="ohp", bufs=16))
        smp = ctx.enter_context(tc.tile_pool(name="smp", bufs=8))
        tmp = ctx.enter_context(tc.tile_pool(name="tmq", bufs=4))
        psA = ctx.enter_context(tc.tile_pool(name="psA", bufs=2, space="PSUM"))
        psT = ctx.enter_context(tc.tile_pool(name="psT", bufs=2, space="PSUM"))
        psD = ctx.enter_context(tc.tile_pool(name="psD", bufs=2, space="PSUM"))

        # constants
        iota = cst.tile([128, 128], bf16)
        nc.sync.dma_start(iota[:], iota_d[:])
        ident = cst.tile([128, 128], bf16)
        nc.sync.dma_start(ident[:], id_d[:])
        gl = cst.tile([128, TLO * 8], i16)
        nc.sync.dma_start(gl[:], gl_d[:])
        gh = cst.tile([128, THI * 8], i16)
        nc.sync.dma_start(gh[:], gh_d[:])
        dlo = cst.tile([128, TLO], f32)
        nc.sync.dma_start(dlo[:], dl_d[:])
        dhi = cst.tile([128, THI], f32)
        nc.sync.dma_start(dhi[:], dh_d[:])
        disT = cst.tile([128, W], bf16)
        nc.sync.dma_start(disT[:], disT_d[:])
        ndis = cst.tile([64, W], bf16)
        nc.sync.dma_start(ndis[:], nd_d[:])
        n2d2 = cst.tile([64, W], bf16)
        nc.sync.dma_start(n2d2[:], n2_d[:])
        Wt = {}
        for k, d in W_d.items():
            Wt[k] = cst.tile(list(d.shape), bf16, name=f"w_{k[0]}_{k[1]}",
                             tag=f"w_{k[0]}_{k[1]}")
            nc.sync.dma_start(Wt[k][:], d[:])

        # state tiles (bf16, feature-major). xt doubles as layer-0 h.
        xt = big.tile([F0, W], bf16, tag="xt")
        nc.sync.dma_start(xt[:], xT_d[:])
        hA = big.tile([64, W], bf16, tag="hA")
        hB = big.tile([64, W], bf16, tag="hB")
        OC = big.tile([64, 2 * W], bf16, tag="OC")  # cols [0,W)=Oa, [W,2W)=Cs1
        ystg = big.tile([128, NT, F2], f32, tag="ystg")

        # ---------------- lap helper
        ohctr = [0]

        def build_oh(dtab, s):
            """[128e,128d] bf16 one-hot: oh[e,d] = (d == dtab[e,s])."""
            oh = ohp.tile([128, 128], bf16, tag="oh")
            ohctr[0] += 1
            if ACT_FRAC > 0 and ohctr[0] % ACT_FRAC == 0:
                t1 = ohp.tile([128, 128], bf16, tag="ohT")
                # (dl - j)^2 then relu(1 - t): exact 0/1 at integer grid
                nc.scalar.activation(t1[:], iota[:], ACTF.Square,
                                     bias=dtab[:, s : s + 1], scale=-1.0)
                nc.scalar.activation(oh[:], t1[:], ACTF.Relu,
                                     bias=1.0, scale=-1.0)
            else:
                nc.vector.tensor_scalar(oh[:], iota[:], dtab[:, s : s + 1],
                                        None, AOT.is_equal)
            return oh

        def lap(agi, epi):
            """acc_T[64f,128d] per dst tile = sum over edges of gathered rows;
            epi(t, acc_psum) consumes the PSUM partial sums."""
            src_lo = ag_out[agi][0:LO, :]
            src_hi = ag_out[agi][LO:NG, :]
            for tl in groups:
                a_lo, b_lo = int(LOFF[tl[0]]), int(LOFF[tl[-1] + 1])
                a_hi, b_hi = int(HOFF[tl[0]]), int(HOFF[tl[-1] + 1])
                nlo, nhi = b_lo - a_lo, b_hi - a_hi
                glo = gp.tile([128, nlo, 128], bf16, tag="glo")
                for o in range(0, nlo, CAP):
                    n = min(CAP, nlo - o)
                    nc.gpsimd.dma_gather(
                        glo[:, o : o + n, :], src_lo,
                        gl[:, (a_lo + o) * 8 : (a_lo + o + n) * 8],
                        num_idxs=n * 128, num_idxs_reg=n * 128, elem_size=128,
                    )
                ghi_t = gp.tile([128, nhi, 128], bf16, tag="ghi")
                for o in range(0, nhi, CAP):
                    n = min(CAP, nhi - o)
                    nc.gpsimd.dma_gather(
                        ghi_t[:, o : o + n, :], src_hi,
                        gh[:, (a_hi + o) * 8 : (a_hi + o + n) * 8],
                        num_idxs=n * 128, num_idxs_reg=n * 128, elem_size=128,
                    )
                for t in tl:
                    acc = psA.tile([64, 128], f32, tag="acc")
                    ntot = int(Klo[t]) + int(Khi[t])
                    i = 0
                    for k in range(int(Klo[t])):
                        s = int(LOFF[t]) + k
                        oh = build_oh(dlo, s)
                        nc.tensor.matmul(
                            acc[:], glo[:, s - a_lo, 0:64], oh[:],
                            start=(i == 0), stop=(i == ntot - 1),
                        )
                        i += 1
                    for k in range(int(Khi[t])):
                        s = int(HOFF[t]) + k
                        oh = build_oh(dhi, s)
                        nc.tensor.matmul(
                            acc[:], ghi_t[:, s - a_hi, 0:64], oh[:],
                            start=(i == 0), stop=(i == ntot - 1),
                        )
                        i += 1
                    epi(t, acc)

        def stage_tile(stage, t, src_bf):
            """src_bf [64,128] bf16 -> node-major bf16 duplicated into stage."""
            ps = psT.tile([128, 64], bf16, tag="pt")
            nc.tensor.transpose(ps[:], src_bf, ident[0:64, 0:64])
            nc.scalar.copy(stage[:, t, 0:64], ps[:])
            nc.scalar.copy(stage[:, t, 64:128], ps[:])

        def do_ag(agi, stage):
            nc.sync.dma_start(
                ag_in[agi].rearrange("(t p) f -> p t f", p=128), stage[:]
            )
            if agmode == "cc":
                nc.gpsimd.collective_compute(
                    "AllGather", mybir.AluOpType.bypass,
                    replica_groups=[list(range(NCORES))],
                    ins=[ag_in[agi][:, :]], outs=[ag_out[agi][:, :]],
                )
            else:
                nc.sync.dma_start(ag_out[agi][0:NPAD, :], ag_in[agi][:, :])

        # ---------------- layers
        for l in range(3):
            Fin = Fins[l]
            hT = xt if l == 0 else hA if l == 1 else hB

            # dense GEMMs + stage As
            stage1 = stg.tile([128, NT, 128], bf16, tag="stage")
            for tl in groups:
                g0 = tl[0] * 128
                w = len(tl) * 128
                # hs = dis * h, built per group (layer-invariant pattern)
                xs = smp.tile([128, 512], bf16, tag="xs")
                nc.vector.tensor_mul(xs[:Fin, :w], hT[:Fin, g0 : g0 + w],
                                     disT[:Fin, g0 : g0 + w])
                hs_ap = xs[:Fin, :w]
                # As = (dis*h) @ wc  -> transpose+stage
                pd = psD.tile([64, 512], f32, tag="pd")
                nc.tensor.matmul(pd[:, :w], Wt["wc", l][:Fin, :], hs_ap)
                asb = tmp.tile([64, 512], bf16, tag="asb")
                nc.scalar.copy(asb[:, :w], pd[:, :w])
                for u, t in enumerate(tl):
                    stage_tile(stage1, t, asb[:, u * 128 : (u + 1) * 128])
                # Cs1 = (dis*h) @ wb
                pd2 = psD.tile([64, 512], f32, tag="pd")
                nc.tensor.matmul(pd2[:, :w], Wt["wb", l][:Fin, :], hs_ap)
                nc.scalar.copy(OC[:, W + g0 : W + g0 + w], pd2[:, :w])
                # Oa = h @ wa
                pd3 = psD.tile([64, 512], f32, tag="pd")
                nc.tensor.matmul(pd3[:, :w], Wt["wa", l][:Fin, :],
                                 hT[:Fin, g0 : g0 + w])
                nc.scalar.copy(OC[:, g0 : g0 + w], pd3[:, :w])

            agA = 2 * l
            do_ag(agA, stage1)

            # lap 1: Cs = Cs1 + n2dis2 * S(As), staged for AG
            stage2 = stg.tile([128, NT, 128], bf16, tag="stage")

            def epi1(t, acc):
                tc0 = t * 128
                ab = smp.tile([64, 128], bf16, tag="ab")
                nc.scalar.copy(ab[:], acc[:])
                t1 = smp.tile([64, 128], bf16, tag="t1")
                nc.vector.tensor_mul(t1[:], ab[:], n2d2[:, tc0 : tc0 + 128])
                cs = smp.tile([64, 128], bf16, tag="cs")
                nc.vector.tensor_add(cs[:], t1[:],
                                     OC[:, W + tc0 : W + tc0 + 128])
                stage_tile(stage2, t, cs[:])

            lap(agA, epi1)

            agC = 2 * l + 1
            do_ag(agC, stage2)

            # lap 2: h' = relu(Oa + negdis * S(Cs) + b)
            hn = hA if l == 0 else hB if l == 1 else None

            def epi2(t, acc):
                tc0 = t * 128
                ab = smp.tile([64, 128], bf16, tag="ab")
                nc.scalar.copy(ab[:], acc[:])
                t1 = smp.tile([64, 128], bf16, tag="t1")
                nc.vector.tensor_mul(t1[:], ab[:], ndis[:, tc0 : tc0 + 128])
                pre = smp.tile([64, 128], bf16, tag="pre")
                nc.vector.tensor_add(pre[:], t1[:], OC[:, tc0 : tc0 + 128])
                if use_bias[l]:
                    pre2 = smp.tile([64, 128], bf16, tag="pre2")
                    nc.vector.tensor_scalar(pre2[:], pre[:],
                                            Wt["br", l][:, 0:1], None, AOT.add)
                    pre_ap = pre2
                else:
                    pre_ap = pre
                if l < 2:
                    nc.vector.tensor_scalar_max(hn[:, tc0 : tc0 + 128],
                                                pre_ap[:], 0.0)
                else:
                    psy = psT.tile([128, F2], bf16, tag="py")
                    nc.tensor.transpose(psy[:], pre_ap[0:F2, :],
                                        ident[0:F2, 0:F2])
                    nc.scalar.copy(ystg[:, t, :], psy[:])

            lap(agC, epi2)

        nc.sync.dma_start(yv[:], ystg[:])

    nc.compile()
    return nc


# ---------------------------------------------------------------- entry
def _run(x, edge_index, Ws, bs, cfg=None, trace=False):
    from concourse.bass_utils import run_bass_kernel_spmd
    from ml_dtypes import bfloat16

    c = _derive(cfg or _REAL)
    N, NCORES, NPC, NPAD = c["N"], c["NCORES"], c["NPC"], c["NPAD"]
    F0, F2, FW, W = c["F0"], c["F2"], c["FW"], c["W"]

    x = np.ascontiguousarray(np.asarray(x, dtype=np.float32))
    pp = _prep(edge_index, c)

    Fins = [F0, FW, FW]
    use_bias = [bool(np.any(b)) for b in bs]
    nc = _build(c, pp, Fins, use_bias)

    iota = np.tile(np.arange(128, dtype=np.float32), (128, 1)).astype(bfloat16)
    ident = np.eye(128, dtype=np.float32).astype(bfloat16)

    def padW(w, fin):
        out = np.zeros((fin, FW), np.float32)
        out[: w.shape[0], : w.shape[1]] = w
        return out.astype(bfloat16)

    base = {"iota": iota, "ident": ident}
    for l in range(3):
        Wl = np.asarray(Ws[l], dtype=np.float32)
        base[f"wa{l}"] = padW(Wl[0] - Wl[2], Fins[l])
        base[f"wb{l}"] = padW(Wl[1], Fins[l])
        base[f"wc{l}"] = padW(Wl[2], Fins[l])
        if use_bias[l]:
            br = np.zeros((64, 1), np.float32)
            br[: bs[l].shape[0], 0] = np.asarray(bs[l], np.float32)
            base[f"br{l}"] = br.astype(bfloat16)

    in_maps = []
    for cc in range(NCORES):
        xl = np.zeros((NPAD, F0), np.float32)
        xl[:NPC] = x[cc * NPC : (cc + 1) * NPC]
        d = pp["dis_pad"][cc]  # [NPAD]
        in_maps.append(
            dict(
                base,
                xT=np.ascontiguousarray(xl.T).astype(bfloat16),
                gidx_lo=pp["gidx_lo"][cc],
                gidx_hi=pp["gidx_hi"][cc],
                dloc_lo=np.ascontiguousarray(pp["dloc_lo"][cc]),
                dloc_hi=np.ascontiguousarray(pp["dloc_hi"][cc]),
                disT=np.broadcast_to(d, (128, NPAD)).astype(bfloat16),
                negdisT=np.broadcast_to(-d, (64, NPAD)).astype(bfloat16),
                n2dis2T=np.broadcast_to(-2.0 * d * d, (64, NPAD)).astype(bfloat16),
            )
        )

    res = run_bass_kernel_spmd(nc, in_maps, core_ids=list(range(NCORES)), trace=trace)
    out = np.concatenate([res.results[cc]["y"][:NPC] for cc in range(NCORES)], axis=0)
    return out[:, :F2], res


def kernel(x, edge_index, W1, b1, Wm, bm, W2, b2):
    out, _ = _run(
        np.asarray(x), np.asarray(edge_index),
        [np.asarray(W1), np.asarray(Wm), np.asarray(W2)],
        [np.asarray(b1), np.asarray(bm), np.asarray(b2)],
    )
    return out


# revision 18
# speedup vs baseline: 1.1226x; 1.0343x over previous
"""ChebNet (K=3, 3 layers) GNN on 8 Trainium2 NeuronCores — v2.

Math per layer: out = h@(W0-W2) + L(h@W1 + 2*L(h@W2)) + b, where
L(v) = -dis * S(dis * v), S = unweighted scatter-add over edges, and
dis = rsqrt(clamp(deg,1)) masked by deg>0 (computed on HOST).

v2 design (vs the one-hot/f32 baseline):
- Feature-major state: h lives as hT [F, nodes] in SBUF; dense GEMMs use
  W as stationary and hT as moving (N=512) with zero transposes; only the
  49 per-tile pre-AllGather transposes remain (PE).
- bf16 everywhere on the lap path: the AllGather table is [node, 128] bf16
  with the 64 features DUPLICATED (256 B rows - dma_gather minimum elem).
- One-hots built in bf16: DVE tensor_scalar is_equal runs in 4x mode
  (~190 ns vs 1476 ns f32 measured); a fraction is built on the idle
  Scalar engine via square/relu trick to overlap.
- Scatter matmul per 128-edge chunk: stationary = gathered rows
  [128e, 64] bf16, moving = one-hot [128e, 128dst] bf16, accumulating
  accT [64f, 128dst] in PSUM (feature-major output, no transpose back).
- Epilogues scale acc by prebuilt per-column tables (negdisT / n2dis2T,
  broadcast down feature partitions, bf16, host-supplied).
- Gathers issued as 2048-index dma_gather calls, many in flight (pool
  bufs), so SDMA transfer overlaps DVE/ACT/PE chunk work.
"""

import os
import sys

sys.path.insert(0, "/opt/trn_rl_repo")

import numpy as np
from contextlib import ExitStack

_REAL = dict(N=50000, E=800000, NCORES=8, LO=32768, F0=128, FW=64, F2=16)


# ---------------------------------------------------------------- host prep
def _derive(cfg):
    c = dict(cfg)
    c["NPC"] = c["N"] // c["NCORES"]
    c["NT"] = -(-c["NPC"] // 128)
    c["NPAD"] = c["NT"] * 128
    c["NG"] = c["NCORES"] * c["NPAD"]
    c["W"] = c["NT"] * 128
    assert c["LO"] <= 32768 and c["NG"] - c["LO"] <= 32768
    assert c["N"] % c["NCORES"] == 0
    return c


def _prep(edge_index, c):
    from ml_dtypes import bfloat16

    N, E, NCORES, LO = c["N"], c["E"], c["NCORES"], c["LO"]
    NPC, NT, NPAD = c["NPC"], c["NT"], c["NPAD"]

    src = np.asarray(edge_index[0], dtype=np.int64)
    dst = np.asarray(edge_index[1], dtype=np.int64)
    assert src.shape == (E,) and dst.shape == (E,)
    psrc = (src // NPC) * NPAD + (src % NPC)  # padded global row of src

    # ---- degrees -> dis on host (reference: deg = segment_sum(ones, src))
    deg = np.bincount(src, minlength=N).astype(np.float32)
    dis = np.where(deg > 0, 1.0 / np.sqrt(np.maximum(deg, 1.0)), 0.0).astype(
        np.float32
    )
    dis_pad = np.zeros((NCORES, NPAD), np.float32)
    for cc in range(NCORES):
        dis_pad[cc, :NPC] = dis[cc * NPC : (cc + 1) * NPC]

    cd = dst // NPC
    ld = dst - cd * NPC
    td = ld >> 7
    dl = ld & 127
    hi = (psrc >= LO).astype(np.int64)

    # ---- lap tables: edges grouped by (core, dst-tile, window), src-sorted
    counts = np.zeros((NCORES, NT, 2), np.int64)
    np.add.at(counts, (cd, td, hi), 1)
    Klo = np.maximum(1, -(-counts[:, :, 0].max(0) // 128))
    Khi = np.maximum(1, -(-counts[:, :, 1].max(0) // 128))
    LOFF = np.concatenate([[0], np.cumsum(Klo)]).astype(np.int64)
    HOFF = np.concatenate([[0], np.cumsum(Khi)]).astype(np.int64)
    TLO, THI = int(LOFF[-1]), int(HOFF[-1])

    order = np.lexsort((psrc, hi, td, cd))
    cd_s, td_s, hi_s = cd[order], td[order], hi[order]
    dl_s, psrc_s = dl[order], psrc[order]
    grp = (cd_s * NT + td_s) * 2 + hi_s
    gc = np.bincount(grp, minlength=NCORES * NT * 2)
    gstart = np.concatenate([[0], np.cumsum(gc)])[:-1]
    rank = np.arange(E) - gstart[grp]

    gidx_lo = np.zeros((NCORES, TLO * 128), np.int16)
    gidx_hi = np.zeros((NCORES, THI * 128), np.int16)
    dloc_lo = np.full((NCORES, 128, TLO), -1.0, np.float32)
    dloc_hi = np.full((NCORES, 128, THI), -1.0, np.float32)
    for cc in range(NCORES):
        for h, (gidx, dloc, OFF, base) in enumerate(
            [(gidx_lo, dloc_lo, LOFF, 0), (gidx_hi, dloc_hi, HOFF, LO)]
        ):
            m = (cd_s == cc) & (hi_s == h)
            slot = OFF[td_s[m]] + rank[m] // 128
            part = rank[m] & 127
            gidx[cc, slot * 128 + part] = (psrc_s[m] - base).astype(np.int16)
            dloc[cc, part, slot] = dl_s[m].astype(np.float32)

    def wrap(a):  # int16 [M*128] -> [128, M*8], idx j at [j%16, j//16], x8 replicated
        return np.tile(a.reshape(-1, 16).T, (8, 1)).copy()

    # ---- host-precomputed one-hots, [128e, T, 128d] bf16 per core
    # (streamed from DRAM via HWDGE; avoids building on DVE, whose SBUF port
    # contends with SWDGE descriptor-ring writes)
    def onehots(dloc):
        ncore, P, T = dloc.shape
        oh = np.zeros((ncore, P, T, 128), bfloat16)
        cc, ee, ss = np.nonzero(dloc >= 0)
        oh[cc, ee, ss, dloc[cc, ee, ss].astype(np.int64)] = 1
        return oh

    oh_lo = onehots(dloc_lo)
    oh_hi = onehots(dloc_hi)
    ohtab = [
        np.ascontiguousarray(
            np.concatenate([oh_lo[cc], oh_hi[cc]], axis=1).reshape(128, -1)
        )
        for cc in range(NCORES)
    ]

    return dict(
        Klo=Klo, Khi=Khi, LOFF=LOFF, HOFF=HOFF, TLO=TLO, THI=THI,
        gidx_lo=[wrap(gidx_lo[cc]) for cc in range(NCORES)],
        gidx_hi=[wrap(gidx_hi[cc]) for cc in range(NCORES)],
        dloc_lo=dloc_lo, dloc_hi=dloc_hi, ohtab=ohtab,
        dis_pad=dis_pad,
    )


# ---------------------------------------------------------------- device build
def _build(c, pp, Fins, use_bias):
    import concourse.bacc as bacc
    import concourse.tile as tile
    from concourse import mybir

    f32, i16, bf16 = mybir.dt.float32, mybir.dt.int16, mybir.dt.bfloat16
    AOT = mybir.AluOpType
    ACTF = mybir.ActivationFunctionType
    NT, NPAD, NG, LO, W = c["NT"], c["NPAD"], c["NG"], c["LO"], c["W"]
    NCORES, F0, FW, F2 = c["NCORES"], c["F0"], c["FW"], c["F2"]
    TLO, THI = pp["TLO"], pp["THI"]
    Klo, Khi = pp["Klo"], pp["Khi"]
    LOFF, HOFF = pp["LOFF"], pp["HOFF"]
    GRP = 4
    groups = [list(range(g * GRP, min((g + 1) * GRP, NT))) for g in range(-(-NT // GRP))]
    CAP = int(os.environ.get("KCAP", "8"))  # 128-idx chunks per call (1024 = ucode max)
    ACT_FRAC = int(os.environ.get("KACT", "4"))  # 1-in-N one-hots built on ScalarE

    nc = bacc.Bacc(num_devices=NCORES)

    xT_d = nc.dram_tensor("xT", [F0, W], bf16, kind="ExternalInput")
    gl_d = nc.dram_tensor("gidx_lo", [128, TLO * 8], i16, kind="ExternalInput")
    gh_d = nc.dram_tensor("gidx_hi", [128, THI * 8], i16, kind="ExternalInput")
    oh_d = nc.dram_tensor("ohtab", [128, (TLO + THI) * 128], bf16,
                          kind="ExternalInput")
    dl_d = nc.dram_tensor("dloc_lo", [128, TLO], f32, kind="ExternalInput")
    dh_d = nc.dram_tensor("dloc_hi", [128, THI], f32, kind="ExternalInput")
    iota_d = nc.dram_tensor("iota", [128, 128], bf16, kind="ExternalInput")
    id_d = nc.dram_tensor("ident", [128, 128], bf16, kind="ExternalInput")
    disT_d = nc.dram_tensor("disT", [128, W], bf16, kind="ExternalInput")
    nd_d = nc.dram_tensor("negdisT", [64, W], bf16, kind="ExternalInput")
    n2_d = nc.dram_tensor("n2dis2T", [64, W], bf16, kind="ExternalInput")
    W_d = {}
    for l in range(3):
        for nm in ("wa", "wb", "wc"):
            W_d[nm, l] = nc.dram_tensor(f"{nm}{l}", [Fins[l], FW], bf16,
                                        kind="ExternalInput")
        if use_bias[l]:
            W_d["br", l] = nc.dram_tensor(f"br{l}", [64, 1], bf16,
                                          kind="ExternalInput")
    y_d = nc.dram_tensor("y", [NPAD, F2], f32, kind="ExternalOutput")
    yv = y_d.rearrange("(t p) f -> p t f", p=128)

    ag_in = [nc.dram_tensor(f"agin{i}", [NPAD, 128], bf16) for i in range(6)]
    ag_out = [nc.dram_tensor(f"agout{i}", [NG, 128], bf16, addr_space="Shared")
              for i in range(6)]
    agmode = os.environ.get("KAG", "cc")

    with tile.TileContext(nc) as tc, ExitStack() as ctx:
        cst = ctx.enter_context(tc.tile_pool(name="cst", bufs=1))
        big = ctx.enter_context(tc.tile_pool(name="big", bufs=1))
        stg = ctx.enter_context(tc.tile_pool(name="stg", bufs=2))
        gp = ctx.enter_context(tc.tile_pool(name="gp", bufs=8))
        ohp = ctx.enter_context(tc.tile_pool(name="ohp", bufs=8))
        smp = ctx.enter_context(tc.tile_pool(name="smp", bufs=8))
        tmp = ctx.enter_context(tc.tile_pool(name="tmq", bufs=4))
        psA = ctx.enter_context(tc.tile_pool(name="psA", bufs=2, space="PSUM"))
        psT = ctx.enter_context(tc.tile_pool(name="psT", bufs=2, space="PSUM"))
        psD = ctx.enter_context(tc.tile_pool(name="psD", bufs=2, space="PSUM"))

        # constants
        iota = cst.tile([128, 128], bf16)
        nc.sync.dma_start(iota[:], iota_d[:])
        ident = cst.tile([128, 128], bf16)
        nc.sync.dma_start(ident[:], id_d[:])
        gl = cst.tile([128, TLO * 8], i16)
        nc.sync.dma_start(gl[:], gl_d[:])
        gh = cst.tile([128, THI * 8], i16)
        nc.sync.dma_start(gh[:], gh_d[:])
        dlo = cst.tile([128, TLO], f32)
        nc.sync.dma_start(dlo[:], dl_d[:])
        dhi = cst.tile([128, THI], f32)
        nc.sync.dma_start(dhi[:], dh_d[:])
        disT = cst.tile([128, W], bf16)
        nc.sync.dma_start(disT[:], disT_d[:])
        ndis = cst.tile([64, W], bf16)
        nc.sync.dma_start(ndis[:], nd_d[:])
        n2d2 = cst.tile([64, W], bf16)
        nc.sync.dma_start(n2d2[:], n2_d[:])
        Wt = {}
        for k, d in W_d.items():
            Wt[k] = cst.tile(list(d.shape), bf16, name=f"w_{k[0]}_{k[1]}",
                             tag=f"w_{k[0]}_{k[1]}")
            nc.sync.dma_start(Wt[k][:], d[:])

        # state tiles (bf16, feature-major). xt doubles as layer-0 h.
        xt = big.tile([F0, W], bf16, tag="xt")
        nc.sync.dma_start(xt[:], xT_d[:])
        hA = big.tile([64, W], bf16, tag="hA")
        hB = big.tile([64, W], bf16, tag="hB")
        OC = big.tile([64, 2 * W], bf16, tag="OC")  # cols [0,W)=Oa, [W,2W)=Cs1
        ystg = big.tile([128, NT, F2], f32, tag="ystg")

        # ---------------- lap helper
        ohctr = [0]

        def build_oh(dtab, s):
            """[128e,128d] bf16 one-hot: oh[e,d] = (d == dtab[e,s])."""
            oh = ohp.tile([128, 128], bf16, tag="oh")
            ohctr[0] += 1
            if ACT_FRAC > 0 and ohctr[0] % ACT_FRAC == 0:
                t1 = ohp.tile([128, 128], bf16, tag="ohT")
                # (dl - j)^2 then relu(1 - t): exact 0/1 at integer grid
                nc.scalar.activation(t1[:], iota[:], ACTF.Square,
                                     bias=dtab[:, s : s + 1], scale=-1.0)
                nc.scalar.activation(oh[:], t1[:], ACTF.Relu,
                                     bias=1.0, scale=-1.0)
            else:
                nc.vector.tensor_scalar(oh[:], iota[:], dtab[:, s : s + 1],
                                        None, AOT.is_equal)
            return oh

        gsem = nc.alloc_semaphore("gsem")
        prep_mode = os.environ.get("KPREP", "0") == "1"
        oh_mode = os.environ.get("KOH", "host")

        def gather_call(dst_ap, src_ap, idx_ap, n):
            if prep_mode:
                # prepare_only: Q7 only generates descriptors; the transfer
                # runs async on the SDMA engines after trigger_dma.
                nc.gpsimd.dma_gather(
                    dst_ap, src_ap, idx_ap,
                    num_idxs=n * 128, num_idxs_reg=n * 128, elem_size=128,
                    prepare_only=True, sem=gsem,
                )
                nc.gpsimd.trigger_dma(count=None)
            else:
                nc.gpsimd.dma_gather(
                    dst_ap, src_ap, idx_ap,
                    num_idxs=n * 128, num_idxs_reg=n * 128, elem_size=128,
                )

        def lap(agi, epi):
            """acc_T[64f,128d] per dst tile = sum over edges of gathered rows;
            epi(t, acc_psum) consumes the PSUM partial sums."""
            srcs = [ag_out[agi][0:LO, :], ag_out[agi][LO:NG, :]]
            idxs = [gl, gh]
            dtabs = [dlo, dhi]
            Ks = [Klo, Khi]
            OFFs = [LOFF, HOFF]
            bases = [0, TLO]  # chunk offset of each window in ohtab
            for t in range(NT):
                acc = psA.tile([64, 128], f32, tag="acc")
                ntot = int(Klo[t]) + int(Khi[t])
                i = 0
                for win in range(2):
                    Kt = int(Ks[win][t])
                    off = int(OFFs[win][t])
                    for k0 in range(0, Kt, CAP):
                        n = min(CAP, Kt - k0)
                        s0 = off + k0
                        g = gp.tile([128, CAP, 128], bf16, tag="g")
                        gather_call(g[:, :n, :], srcs[win],
                                    idxs[win][:, s0 * 8 : (s0 + n) * 8], n)
                        if oh_mode == "host":
                            oh = ohp.tile([128, CAP * 128], bf16, tag="oh")
                            c0 = (bases[win] + s0) * 128
                            nc.sync.dma_start(oh[:, : n * 128],
                                              oh_d[:, c0 : c0 + n * 128])
                            for k in range(n):
                                nc.tensor.matmul(
                                    acc[:], g[:, k, 0:64],
                                    oh[:, k * 128 : (k + 1) * 128],
                                    start=(i == 0), stop=(i == ntot - 1),
                                )
                                i += 1
                        else:
                            for k in range(n):
                                oh = build_oh(dtabs[win], s0 + k)
                                nc.tensor.matmul(
                                    acc[:], g[:, k, 0:64], oh[:],
                                    start=(i == 0), stop=(i == ntot - 1),
                                )
                                i += 1
                epi(t, acc)

        def stage_tile(stage, t, src_bf):
            """src_bf [64,128] bf16 -> node-major bf16 duplicated into stage."""
            ps = psT.tile([128, 64], bf16, tag="pt")
            nc.tensor.transpose(ps[:], src_bf, ident[0:64, 0:64])
            nc.scalar.copy(stage[:, t, 0:64], ps[:])
            nc.scalar.copy(stage[:, t, 64:128], ps[:])

        def do_ag(agi, stage):
            nc.sync.dma_start(
                ag_in[agi].rearrange("(t p) f -> p t f", p=128), stage[:]
            )
            if agmode == "cc":
                nc.gpsimd.collective_compute(
                    "AllGather", mybir.AluOpType.bypass,
                    replica_groups=[list(range(NCORES))],
                    ins=[ag_in[agi][:, :]], outs=[ag_out[agi][:, :]],
                )
            else:
                nc.sync.dma_start(ag_out[agi][0:NPAD, :], ag_in[agi][:, :])

        # ---------------- layers
        for l in range(3):
            Fin = Fins[l]
            hT = xt if l == 0 else hA if l == 1 else hB

            # dense GEMMs + stage As
            stage1 = stg.tile([128, NT, 128], bf16, tag="stage")
            for tl in groups:
                g0 = tl[0] * 128
                w = len(tl) * 128
                # hs = dis * h, built per group (layer-invariant pattern)
                xs = smp.tile([128, 512], bf16, tag="xs")
                nc.vector.tensor_mul(xs[:Fin, :w], hT[:Fin, g0 : g0 + w],
                                     disT[:Fin, g0 : g0 + w])
                hs_ap = xs[:Fin, :w]
                # As = (dis*h) @ wc  -> transpose+stage
                pd = psD.tile([64, 512], f32, tag="pd")
                nc.tensor.matmul(pd[:, :w], Wt["wc", l][:Fin, :], hs_ap)
                asb = tmp.tile([64, 512], bf16, tag="asb")
                nc.scalar.copy(asb[:, :w], pd[:, :w])
                for u, t in enumerate(tl):
                    stage_tile(stage1, t, asb[:, u * 128 : (u + 1) * 128])
                # Cs1 = (dis*h) @ wb
                pd2 = psD.tile([64, 512], f32, tag="pd")
                nc.tensor.matmul(pd2[:, :w], Wt["wb", l][:Fin, :], hs_ap)
                nc.scalar.copy(OC[:, W + g0 : W + g0 + w], pd2[:, :w])
                # Oa = h @ wa
                pd3 = psD.tile([64, 512], f32, tag="pd")
                nc.tensor.matmul(pd3[:, :w], Wt["wa", l][:Fin, :],
                                 hT[:Fin, g0 : g0 + w])
                nc.scalar.copy(OC[:, g0 : g0 + w], pd3[:, :w])

            agA = 2 * l
            do_ag(agA, stage1)

            # lap 1: Cs = Cs1 + n2dis2 * S(As), staged for AG
            stage2 = stg.tile([128, NT, 128], bf16, tag="stage")

            def epi1(t, acc):
                tc0 = t * 128
                ab = smp.tile([64, 128], bf16, tag="ab")
                nc.scalar.copy(ab[:], acc[:])
                t1 = smp.tile([64, 128], bf16, tag="t1")
                nc.vector.tensor_mul(t1[:], ab[:], n2d2[:, tc0 : tc0 + 128])
                cs = smp.tile([64, 128], bf16, tag="cs")
                nc.vector.tensor_add(cs[:], t1[:],
                                     OC[:, W + tc0 : W + tc0 + 128])
                stage_tile(stage2, t, cs[:])

            lap(agA, epi1)

            agC = 2 * l + 1
            do_ag(agC, stage2)

            # lap 2: h' = relu(Oa + negdis * S(Cs) + b)
            hn = hA if l == 0 else hB if l == 1 else None

            def epi2(t, acc):
                tc0 = t * 128
                ab = smp.tile([64, 128], bf16, tag="ab")
                nc.scalar.copy(ab[:], acc[:])
                t1 = smp.tile([64, 128], bf16, tag="t1")
                nc.vector.tensor_mul(t1[:], ab[:], ndis[:, tc0 : tc0 + 128])
                pre = smp.tile([64, 128], bf16, tag="pre")
                nc.vector.tensor_add(pre[:], t1[:], OC[:, tc0 : tc0 + 128])
                if use_bias[l]:
                    pre2 = smp.tile([64, 128], bf16, tag="pre2")
                    nc.vector.tensor_scalar(pre2[:], pre[:],
                                            Wt["br", l][:, 0:1], None, AOT.add)
                    pre_ap = pre2
                else:
                    pre_ap = pre
                if l < 2:
                    nc.vector.tensor_scalar_max(hn[:, tc0 : tc0 + 128],
                                                pre_ap[:], 0.0)
                else:
                    psy = psT.tile([128, F2], bf16, tag="py")
                    nc.tensor.transpose(psy[:], pre_ap[0:F2, :],
                                        ident[0:F2, 0:F2])
                    nc.scalar.copy(ystg[:, t, :], psy[:])

            lap(agC, epi2)

        nc.sync.dma_start(yv[:], ystg[:])

    nc.compile()
    return nc


# ---------------------------------------------------------------- entry
def _run(x, edge_index, Ws, bs, cfg=None, trace=False):
    from concourse.bass_utils import run_bass_kernel_spmd
    from ml_dtypes import bfloat16

    c = _derive(cfg or _REAL)
    N, NCORES, NPC, NPAD = c["N"], c["NCORES"], c["NPC"], c["NPAD"]
    F0, F2, FW, W = c["F0"], c["F2"], c["FW"], c["W"]

    x = np.ascontiguousarray(np.asarray(x, dtype=np.float32))
    pp = _prep(edge_index, c)

    Fins = [F0, FW, FW]
    use_bias = [bool(np.any(b)) for b in bs]
    nc = _build(c, pp, Fins, use_bias)

    iota = np.tile(np.arange(128, dtype=np.float32), (128, 1)).astype(bfloat16)
    ident = np.eye(128, dtype=np.float32).astype(bfloat16)

    def padW(w, fin):
        out = np.zeros((fin, FW), np.float32)
        out[: w.shape[0], : w.shape[1]] = w
        return out.astype(bfloat16)

    base = {"iota": iota, "ident": ident}
    for l in range(3):
        Wl = np.asarray(Ws[l], dtype=np.float32)
        base[f"wa{l}"] = padW(Wl[0] - Wl[2], Fins[l])
        base[f"wb{l}"] = padW(Wl[1], Fins[l])
        base[f"wc{l}"] = padW(Wl[2], Fins[l])
        if use_bias[l]:
            br = np.zeros((64, 1), np.float32)
            br[: bs[l].shape[0], 0] = np.asarray(bs[l], np.float32)
            base[f"br{l}"] = br.astype(bfloat16)

    in_maps = []
    for cc in range(NCORES):
        xl = np.zeros((NPAD, F0), np.float32)
        xl[:NPC] = x[cc * NPC : (cc + 1) * NPC]
        d = pp["dis_pad"][cc]  # [NPAD]
        in_maps.append(
            dict(
                base,
                xT=np.ascontiguousarray(xl.T).astype(bfloat16),
                gidx_lo=pp["gidx_lo"][cc],
                gidx_hi=pp["gidx_hi"][cc],
                ohtab=pp["ohtab"][cc],
                dloc_lo=np.ascontiguousarray(pp["dloc_lo"][cc]),
                dloc_hi=np.ascontiguousarray(pp["dloc_hi"][cc]),
                disT=np.broadcast_to(d, (128, NPAD)).astype(bfloat16),
                negdisT=np.broadcast_to(-d, (64, NPAD)).astype(bfloat16),
                n2dis2T=np.broadcast_to(-2.0 * d * d, (64, NPAD)).astype(bfloat16),
            )
        )

    res = run_bass_kernel_spmd(nc, in_maps, core_ids=list(range(NCORES)), trace=trace)
    out = np.concatenate([res.results[cc]["y"][:NPC] for cc in range(NCORES)], axis=0)
    return out[:, :F2], res


def kernel(x, edge_index, W1, b1, Wm, bm, W2, b2):
    out, _ = _run(
        np.asarray(x), np.asarray(edge_index),
        [np.asarray(W1), np.asarray(Wm), np.asarray(W2)],
        [np.asarray(b1), np.asarray(bm), np.asarray(b2)],
    )
    return out
